# revision 17
# baseline (speedup 1.0000x reference)
"""MinGRU Trainium2 kernel (v3 — bf16 inputs + schedule-optimized).

Reference computation (per batch b):
    c = depthwise_conv1d(x, conv_w, taps=5, pad=2)        # [D, L]
    h = h_w @ c                                           # [O, L]
    g = concat([-1000, +1000], g_w @ c)                   # [O, L]
    a = sigmoid(-g); v = sigmoid(g) * h
    out[l] = a[l] * out[l-1] + v[l]     (linear scan along L)

Strategy: pure data-parallel over B (8 batches -> 8 NeuronCores).
Per core, everything streams in PAIRS of 512-wide l-chunks:
  - x / conv diagonals / h_w / g_w / c are bf16 (f32 PSUM accumulate;
    measured end-to-end rel err ~4e-3 vs the 2e-2 budget); activations,
    scan and output stay f32.  bf16 also enables the PE fast-weight-load
    path, so LDWEIGHTS fully hides under the matmul stream.
  - conv: 5 diagonal-matmuls per d-tile on TensorE accumulating in PSUM,
    taps interleaved across the pair's two chunks (one x DMA covers the
    whole pair per d-tile)
  - c PSUM->SBUF copies (cast to bf16) on ScalarE; h/g matmuls
    dt-interleaved across the pair so each stationary weight serves two
    512-col streams
  - a = sigmoid(-(g+bias)) on ScalarE (bias carries the +/-1000 rows)
  - z = 1 - a on GpSimd, v = z*h on VectorE, scan via tensor_tensor_scan
  - DMA issue cost is ~0.6us of sequencer time per dma_start, so:
    Sync issues x loads + half the stores, GpSimd (SWDGE) issues
    weights + small constants + the other half of the stores, Scalar
    issues nothing (its sequencer is saturated by copies + sigmoids).
    Stores are issued one pair late so they never head-of-line block;
    the last pair's stores are split across all three queues.
  - channel 0 output is exactly 0 (output buffers are pre-zeroed)
  - channel 1 replicates the reference's f32 log-domain quantization
    out[1,l] = sign(h)*exp(fl(fl(K_l+ln|h|)-K_l)), K_l = 1000(l+1),
    via a packed [128,64] tile (see emit_endpass).
"""

import numpy as np

import concourse.bass as bass
import concourse.mybir as mybir
from concourse import bacc
from concourse.tile import TileContext
from concourse.bass_utils import run_bass_kernel_spmd

F32 = mybir.dt.float32
BF16 = mybir.dt.bfloat16
U32 = mybir.dt.uint32
AF = mybir.ActivationFunctionType
OP = mybir.AluOpType

B, D, O, L = 8, 512, 512, 4096
P = 128
CH = 512                 # l-chunk width (one PSUM bank)
CH2 = 2 * CH             # pair width
NCH = L // CH            # 8
NPAIR = NCH // 2         # 4
NDT = D // P             # 4 d-tiles
NOT = O // P             # 4 o-tiles
NTAPS = 5
N_CORES = 8
PK = CH // 64            # 8 packed columns per chunk


def build_program():
    nc = bacc.Bacc()

    x = nc.declare_dram_parameter("x", [D, L], BF16, isOutput=False)
    hwT = nc.declare_dram_parameter("hwT", [D, O], BF16, isOutput=False)
    gwT = nc.declare_dram_parameter("gwT", [D, O], BF16, isOutput=False)
    cwdiag = nc.declare_dram_parameter("cwdiag", [D, NTAPS * P], BF16,
                                       isOutput=False)
    gbn = nc.declare_dram_parameter("gbn", [O, 1], F32, isOutput=False)
    kpack = nc.declare_dram_parameter("kpack", [16, 256], F32, isOutput=False)
    zpad = nc.declare_dram_parameter("zpad", [P, CH], BF16, isOutput=False)
    masks = nc.declare_dram_parameter("masks", [P, 2], U32, isOutput=False)
    out = nc.declare_dram_parameter("out", [O, L], F32, isOutput=True)

    with TileContext(nc) as tc:
        with (
            tc.tile_pool(name="weights", bufs=1) as wpool,
            tc.tile_pool(name="xin", bufs=8) as xpool,
            tc.tile_pool(name="csb", bufs=16) as cpool,
            tc.tile_pool(name="actout", bufs=6) as apool,
            tc.tile_pool(name="vtiles", bufs=4) as vpool,
            tc.tile_pool(name="outt", bufs=4) as opool,
            tc.tile_pool(name="cps", bufs=3, space="PSUM") as cps_pool,
            tc.tile_pool(name="hps", bufs=2, space="PSUM") as hps_pool,
            tc.tile_pool(name="gps", bufs=3, space="PSUM") as gps_pool,
        ):
            # ---- constants / weights --------------------------------------
            # Sync: zpad (warm-up + halos), then cw diagonals interleaved
            # with the pair-0/1 x tiles.  GpSimd SWDGE: small constants and
            # the h/g weights (its Q7 is idle until the first rest-block).
            # Scalar issues no DMAs at all.
            wz_sb = wpool.tile([P, CH], BF16, tag="wz")
            nc.sync.dma_start(out=wz_sb, in_=zpad[:, :])
            cw_sb = [wpool.tile([P, NTAPS * P], BF16, tag=f"cw{dt}",
                                name=f"cw{dt}")
                     for dt in range(NDT)]

            gbn_sb = [wpool.tile([P, 1], F32, tag=f"gbn{ot}", name=f"gbn{ot}")
                      for ot in range(NOT)]
            kpack_sb = wpool.tile([16, 256], F32, tag="kpack")
            masks_sb = wpool.tile([P, 2], U32, tag="masks")
            hwT_sb = [wpool.tile([P, O], BF16, tag=f"hwT{dt}", name=f"hwT{dt}")
                      for dt in range(NDT)]
            gwT_sb = [wpool.tile([P, O], BF16, tag=f"gwT{dt}", name=f"gwT{dt}")
                      for dt in range(NDT)]

            c_sb = [None] * NCH          # [chunk] -> list of 4 SBUF c tiles
            prev_out = [None] * NOT      # previous chunk's out tile per o-tile
            all_ott = [[None] * NOT for _ in range(NCH)]  # for late stores
            hrow = [None] * NCH          # [chunk] -> [2, CH] copy of h rows 0:2
            # h row 1 packed so partition q holds l in [q*256, (q+1)*256):
            # the final channel-1 store is then one DMA of 16 x 1KB
            # contiguous runs (no HBM read-modify-write penalty)
            hpack = wpool.tile([16, 256], F32, tag="hpack")

            def load_xt_pair(pr, dt):
                # one DMA covers both chunks of the pair: cols
                # [pr*1024-2, pr*1024+1026) of x, halo zero-filled from zpad
                lo = pr * CH2
                xt = xpool.tile([P, CH2 + 4], BF16, tag="xt")
                if pr == 0:
                    nc.sync.dma_start(out=xt[:, 0:2], in_=zpad[:, 0:2])
                    nc.sync.dma_start(out=xt[:, 2:CH2 + 4],
                                      in_=x[dt * P:(dt + 1) * P, 0:CH2 + 2])
                elif pr == NPAIR - 1:
                    nc.sync.dma_start(out=xt[:, CH2 + 2:CH2 + 4],
                                      in_=zpad[:, 0:2])
                    nc.sync.dma_start(out=xt[:, 0:CH2 + 2],
                                      in_=x[dt * P:(dt + 1) * P,
                                            lo - 2:lo + CH2])
                else:
                    nc.sync.dma_start(out=xt[:, :],
                                      in_=x[dt * P:(dt + 1) * P,
                                            lo - 2:lo + CH2 + 2])
                return xt

            def emit_conv_pair(pr, cw_dmas=()):
                """conv for chunks (2*pr, 2*pr+1), tap-paired across chunks."""
                i0, i1 = 2 * pr, 2 * pr + 1
                cw_dmas = list(cw_dmas)
                xts = []
                for dt in range(NDT):
                    if cw_dmas:
                        cw_dmas.pop(0)()
                    xts.append(load_xt_pair(pr, dt))
                tap_order = (2, 0, 1, 3, 4)
                tiles0, tiles1 = [], []
                for dt in range(NDT):
                    cp0 = cps_pool.tile([P, CH], F32, tag="cps")
                    cp1 = cps_pool.tile([P, CH], F32, tag="cps")
                    for j, k in enumerate(tap_order):
                        lw = cw_sb[dt][:, k * P:(k + 1) * P]
                        nc.tensor.matmul(cp0, lhsT=lw,
                                         rhs=xts[dt][:, k:k + CH],
                                         start=(j == 0), stop=(j == NTAPS - 1))
                        nc.tensor.matmul(cp1, lhsT=lw,
                                         rhs=xts[dt][:, CH + k:CH2 + k],
                                         start=(j == 0), stop=(j == NTAPS - 1))
                    ct0 = cpool.tile([P, CH], BF16, tag="ct")
                    nc.scalar.copy(ct0, cp0)
                    ct1 = cpool.tile([P, CH], BF16, tag="ct")
                    nc.scalar.copy(ct1, cp1)
                    tiles0.append(ct0)
                    tiles1.append(ct1)
                c_sb[i0] = tiles0
                c_sb[i1] = tiles1

            store_q = [nc.sync, nc.gpsimd, nc.scalar]

            def emit_store(i, ot, q):
                lo = i * CH
                ott = all_ott[i][ot]
                if ot == 0:
                    # rows 0/1 are produced by the end-pass / pre-zeroing
                    q.dma_start(out=out[2:P, lo:lo + CH], in_=ott[2:P, :])
                else:
                    q.dma_start(out=out[ot * P:(ot + 1) * P, lo:lo + CH],
                                in_=ott)

            def emit_rest_pair(pr):
                """h/g + activation chain + scan for chunks (2*pr, 2*pr+1).

                Stores for pair pr-1 are issued first (their scans finished
                a pair ago, so the queues never block on them), split
                Sync/GpSimd; the final pair's stores are issued inline,
                split across all three queues."""
                i0, i1 = 2 * pr, 2 * pr + 1
                last = (pr == NPAIR - 1)
                if pr > 0:
                    for j, (i, ot) in enumerate(
                            [(2 * pr - 2, t) for t in range(NOT)]
                            + [(2 * pr - 1, t) for t in range(NOT)]):
                        emit_store(i, ot, store_q[j % 2])
                for ot in range(NOT):
                    # g before h: the sigmoid chain (ACT) only needs g, so it
                    # starts while the h matmuls are still streaming
                    gp0 = gps_pool.tile([P, CH], F32, tag="gps")
                    gp1 = gps_pool.tile([P, CH], F32, tag="gps")
                    for dt in range(NDT):
                        lw = gwT_sb[dt][:, ot * P:(ot + 1) * P]
                        nc.tensor.matmul(gp0, lhsT=lw, rhs=c_sb[i0][dt],
                                         start=(dt == 0), stop=(dt == NDT - 1))
                        nc.tensor.matmul(gp1, lhsT=lw, rhs=c_sb[i1][dt],
                                         start=(dt == 0), stop=(dt == NDT - 1))
                    hp0 = hps_pool.tile([P, CH], F32, tag="hps")
                    hp1 = hps_pool.tile([P, CH], F32, tag="hps")
                    for dt in range(NDT):
                        lw = hwT_sb[dt][:, ot * P:(ot + 1) * P]
                        nc.tensor.matmul(hp0, lhsT=lw, rhs=c_sb[i0][dt],
                                         start=(dt == 0), stop=(dt == NDT - 1))
                        nc.tensor.matmul(hp1, lhsT=lw, rhs=c_sb[i1][dt],
                                         start=(dt == 0), stop=(dt == NDT - 1))
                    for ci, (i, gp, hp) in enumerate(
                            [(i0, gp0, hp0), (i1, gp1, hp1)]):
                        # a = sigmoid(-(g + bias)) ; z = 1 - a ; v = z * h
                        # the very last chunk runs the chain in 256-col
                        # halves so its serial tail after the final matmul
                        # is half as long
                        halves = ((0, CH),)
                        if last and ci == 1:
                            halves = ((0, CH // 2), (CH // 2, CH))
                        at = apool.tile([P, CH], F32, tag="at")
                        zt = vpool.tile([P, CH], F32, tag="zt")
                        vt = vpool.tile([P, CH], F32, tag="vt")
                        ott = opool.tile([P, CH], F32, tag=f"out{ot}")
                        for lo, hi in halves:
                            nc.scalar.activation(at[:, lo:hi], gp[:, lo:hi],
                                                 AF.Sigmoid,
                                                 bias=gbn_sb[ot], scale=-1.0)
                            nc.gpsimd.tensor_scalar(zt[:, lo:hi], at[:, lo:hi],
                                                    -1.0, 1.0, OP.mult, OP.add)
                            nc.vector.tensor_tensor(vt[:, lo:hi], zt[:, lo:hi],
                                                    hp[:, lo:hi], OP.mult)
                            if lo == 0:
                                init = (0.0 if i == 0
                                        else prev_out[ot][:, CH - 1:CH])
                            else:
                                init = ott[:, lo - 1:lo]
                            nc.vector.tensor_tensor_scan(
                                ott[:, lo:hi], at[:, lo:hi], vt[:, lo:hi],
                                init, OP.mult, OP.add)
                        if ot == 0 and hrow[i] is None:
                            # stash h row 1: hpack[2i+p, j] = h[1, i*512+p*256+j]
                            ht = wpool.tile([2, CH], F32, tag=f"hrow{i}")
                            nc.vector.tensor_copy(ht, hp[0:2, :])
                            nc.gpsimd.dma_start(
                                out=hpack[2 * i:2 * i + 2, :],
                                in_=ht[1:2, :].rearrange("r (p j) -> r p j",
                                                         j=256))
                            hrow[i] = ht
                        all_ott[i][ot] = ott
                        prev_out[ot] = ott
                        if last:
                            emit_store(i, ot, store_q[(2 * ot + ci) % 3])

            def emit_hrow_early(i):
                # h rows 0:2 for chunk i via a tiny 2-row matmul so the
                # end-pass doesn't have to wait for the full h of the last
                # chunks.
                cpx = cps_pool.tile([P, CH], F32, tag="cps", name=f"cpx{i}")
                for dt in range(NDT):
                    nc.tensor.matmul(
                        cpx[0:2, :],
                        lhsT=hwT_sb[dt][:, 0:2],
                        rhs=c_sb[i][dt],
                        start=(dt == 0), stop=(dt == NDT - 1),
                    )
                ht = wpool.tile([2, CH], F32, tag=f"hrow{i}", name=f"hrowE{i}")
                nc.vector.tensor_copy(ht, cpx[0:2, :])
                nc.gpsimd.dma_start(
                    out=hpack[2 * i:2 * i + 2, :],
                    in_=ht[1:2, :].rearrange("r (p j) -> r p j", j=256))
                hrow[i] = ht

            def emit_endpass():
                # ---- channel 1 on the packed [16, 256] tile ----
                # replicates the reference's f32 rounding:
                # out[1,l] = sign(h)*exp(fl(fl(K+ln|h|) - K)), K = 1000(l+1).
                absm = masks_sb[0:16, 0:1]
                sgnm = masks_sb[0:16, 1:2]
                t = wpool.tile([16, 256], F32, tag="ch1w", name="ch1w")
                nc.vector.tensor_scalar(t.bitcast(U32), hpack.bitcast(U32),
                                        absm, None, OP.bitwise_and)
                nc.vector.tensor_scalar_max(t, t, 1e-6)
                nc.scalar.activation(t, t, AF.Ln)
                nc.vector.tensor_tensor(t, t, kpack_sb, OP.add)
                nc.vector.tensor_tensor(t, t, kpack_sb, OP.subtract)
                nc.scalar.activation(t, t, AF.Exp)
                res = wpool.tile([16, 256], F32, tag="ch1r", name="ch1r")
                nc.vector.tensor_scalar(res.bitcast(U32), hpack.bitcast(U32),
                                        sgnm, None, OP.bitwise_and)
                nc.vector.tensor_tensor(res.bitcast(U32), res.bitcast(U32),
                                        t.bitcast(U32), OP.bitwise_or)
                # row 1 in one DMA: partition q -> out[1, q*256:(q+1)*256]
                nc.sync.dma_start(
                    out=out[1:2, :].rearrange("r (q j) -> r q j", j=256),
                    in_=res)

            # ---- prologue: warm-up + conv pair 0 --------------------------
            # PE warm-up: dummy matmuls on the zero tile during the initial
            # DMA wait trip the HAM clock gate to 2.4 GHz before real work
            # arrives; a few [2,512] streams keep it busy until x lands.
            wps = cps_pool.tile([P, CH], F32, tag="cps", name="warmps")
            for _ in range(28):
                nc.tensor.matmul(wps[0:2, 0:2], lhsT=wz_sb[:, 0:2],
                                 rhs=wz_sb[:, 0:2], start=True, stop=True)
            for _ in range(3):
                nc.tensor.matmul(wps[0:2, :], lhsT=wz_sb[:, 0:2],
                                 rhs=wz_sb, start=True, stop=True)
            wout = wpool.tile([2, 2], F32, tag="warmout")
            nc.vector.tensor_copy(wout, wps[0:2, 0:2])

            def _dma_cw(dt):
                return lambda: nc.sync.dma_start(
                    out=cw_sb[dt], in_=cwdiag[dt * P:(dt + 1) * P, :])

            # small constants + h/g weights on the SWDGE queue
            for ot in range(NOT):
                nc.gpsimd.dma_start(out=gbn_sb[ot],
                                    in_=gbn[ot * P:(ot + 1) * P, :])
            nc.gpsimd.dma_start(out=kpack_sb, in_=kpack[:, :])
            nc.gpsimd.dma_start(out=masks_sb, in_=masks[:, :])

            emit_conv_pair(0, cw_dmas=[_dma_cw(0), _dma_cw(1),
                                       _dma_cw(2), _dma_cw(3)])

            for dt in range(NDT):
                nc.gpsimd.dma_start(out=gwT_sb[dt],
                                    in_=gwT[dt * P:(dt + 1) * P, :])
                nc.gpsimd.dma_start(out=hwT_sb[dt],
                                    in_=hwT[dt * P:(dt + 1) * P, :])
            nc.gpsimd.dma_start(out=out[2:4, 0:2], in_=wout)

            # ---- main pipeline, one conv pair ahead -----------------------
            emit_conv_pair(1)
            emit_rest_pair(0)
            emit_conv_pair(2)
            emit_rest_pair(1)
            emit_conv_pair(3)
            emit_hrow_early(NCH - 2)
            emit_hrow_early(NCH - 1)
            emit_rest_pair(2)
            emit_endpass()
            emit_rest_pair(3)

    nc.finalize()
    return nc


_PROGRAM = None


def _get_program():
    global _PROGRAM
    if _PROGRAM is None:
        _PROGRAM = build_program()
    return _PROGRAM


def _bf16(a):
    # round-to-nearest-even f32 -> bf16, returned as uint16-packed bfloat16
    import ml_dtypes
    return np.asarray(a, np.float32).astype(ml_dtypes.bfloat16)


def prepare_in_maps(x, conv_w, h_w, g_w):
    x = np.ascontiguousarray(np.asarray(x), dtype=np.float32)
    conv_w = np.asarray(conv_w, dtype=np.float32)
    h_w = np.asarray(h_w, dtype=np.float32)
    g_w = np.asarray(g_w, dtype=np.float32)

    hwT = np.ascontiguousarray(h_w[:, :, 0].T)                    # [D, O]
    gw_pad = np.zeros((O, D), np.float32)
    gw_pad[2:, :] = g_w[:, :, 0]
    gwT = np.ascontiguousarray(gw_pad.T)                          # [D, O]

    # 5 diagonal matrices per d-tile, concatenated along free dim: [D, 5*128]
    cwdiag = np.zeros((D, NTAPS * P), np.float32)
    for dt in range(NDT):
        for k in range(NTAPS):
            blk = cwdiag[dt * P:(dt + 1) * P, k * P:(k + 1) * P]
            np.fill_diagonal(blk, conv_w[dt * P:(dt + 1) * P, 0, k])

    gbp = np.zeros((O, 1), np.float32)
    gbp[0, 0], gbp[1, 0] = -1000.0, 1000.0
    gbn = np.ascontiguousarray(-gbp)

    # K for the packed layout: kpack[q, j] = 1000*(q*256 + j + 1)
    q = np.arange(16)[:, None]
    j = np.arange(256)[None, :]
    kpack = np.ascontiguousarray(
        (1000.0 * (q * 256 + j + 1.0)).astype(np.float32))        # [16, 256]

    zpad = np.zeros((P, CH), np.float32)
    masks = np.ascontiguousarray(np.broadcast_to(
        np.array([[0x7FFFFFFF, 0x80000000]], np.uint32), (P, 2)))
    xb = _bf16(x)
    return [
        {"x": np.ascontiguousarray(xb[b]), "hwT": _bf16(hwT),
         "gwT": _bf16(gwT), "cwdiag": _bf16(cwdiag),
         "gbn": gbn, "kpack": kpack, "zpad": _bf16(zpad), "masks": masks}
        for b in range(B)
    ]


def kernel(x, conv_w, h_w, g_w):
    in_maps = prepare_in_maps(x, conv_w, h_w, g_w)
    nc = _get_program()
    res = run_bass_kernel_spmd(nc, in_maps, list(range(N_CORES))).results
    return np.stack([res[b]["out"] for b in range(B)], axis=0)


# revision 21
# speedup vs baseline: 1.0043x; 1.0043x over previous
"""MinGRU Trainium2 kernel (v3 — bf16 inputs + schedule-optimized).

Reference computation (per batch b):
    c = depthwise_conv1d(x, conv_w, taps=5, pad=2)        # [D, L]
    h = h_w @ c                                           # [O, L]
    g = concat([-1000, +1000], g_w @ c)                   # [O, L]
    a = sigmoid(-g); v = sigmoid(g) * h
    out[l] = a[l] * out[l-1] + v[l]     (linear scan along L)

Strategy: pure data-parallel over B (8 batches -> 8 NeuronCores).
Per core, everything streams in PAIRS of 512-wide l-chunks:
  - x / conv diagonals / h_w / g_w / c are bf16 (f32 PSUM accumulate;
    measured end-to-end rel err ~4e-3 vs the 2e-2 budget); activations,
    scan and output stay f32.  bf16 also enables the PE fast-weight-load
    path, so LDWEIGHTS fully hides under the matmul stream.
  - conv: 5 diagonal-matmuls per d-tile on TensorE accumulating in PSUM,
    taps interleaved across the pair's two chunks (one x DMA covers the
    whole pair per d-tile)
  - c PSUM->SBUF copies (cast to bf16) on ScalarE; h/g matmuls
    dt-interleaved across the pair so each stationary weight serves two
    512-col streams
  - a = sigmoid(-(g+bias)) on ScalarE (bias carries the +/-1000 rows)
  - z = 1 - a on GpSimd, v = z*h on VectorE, scan via tensor_tensor_scan
  - DMA issue cost is ~0.6us of sequencer time per dma_start, so:
    Sync issues x loads + half the stores, GpSimd (SWDGE) issues
    weights + small constants + the other half of the stores, Scalar
    issues nothing (its sequencer is saturated by copies + sigmoids).
    Stores are issued one pair late so they never head-of-line block;
    the last pair's stores are split across all three queues.
  - channel 0 output is exactly 0 (output buffers are pre-zeroed)
  - channel 1 replicates the reference's f32 log-domain quantization
    out[1,l] = sign(h)*exp(fl(fl(K_l+ln|h|)-K_l)), K_l = 1000(l+1),
    via a packed [128,64] tile (see emit_endpass).
"""

import numpy as np

import concourse.bass as bass
import concourse.mybir as mybir
from concourse import bacc
from concourse.tile import TileContext
from concourse.bass_utils import run_bass_kernel_spmd

F32 = mybir.dt.float32
BF16 = mybir.dt.bfloat16
U32 = mybir.dt.uint32
AF = mybir.ActivationFunctionType
OP = mybir.AluOpType

B, D, O, L = 8, 512, 512, 4096
P = 128
CH = 512                 # l-chunk width (one PSUM bank)
CH2 = 2 * CH             # pair width
NCH = L // CH            # 8
NPAIR = NCH // 2         # 4
NDT = D // P             # 4 d-tiles
NOT = O // P             # 4 o-tiles
NTAPS = 5
N_CORES = 8
PK = CH // 64            # 8 packed columns per chunk


def build_program():
    nc = bacc.Bacc()

    x = nc.declare_dram_parameter("x", [D, L], BF16, isOutput=False)
    hwT = nc.declare_dram_parameter("hwT", [D, O], BF16, isOutput=False)
    gwT = nc.declare_dram_parameter("gwT", [D, O], BF16, isOutput=False)
    cwdiag = nc.declare_dram_parameter("cwdiag", [D, NTAPS * P], BF16,
                                       isOutput=False)
    gbn = nc.declare_dram_parameter("gbn", [O, 1], F32, isOutput=False)
    kpack = nc.declare_dram_parameter("kpack", [16, 256], F32, isOutput=False)
    zpad = nc.declare_dram_parameter("zpad", [P, CH], BF16, isOutput=False)
    masks = nc.declare_dram_parameter("masks", [P, 2], U32, isOutput=False)
    out = nc.declare_dram_parameter("out", [O, L], F32, isOutput=True)

    with TileContext(nc) as tc:
        with (
            tc.tile_pool(name="weights", bufs=1) as wpool,
            tc.tile_pool(name="xin", bufs=8) as xpool,
            tc.tile_pool(name="csb", bufs=16) as cpool,
            tc.tile_pool(name="actout", bufs=6) as apool,
            tc.tile_pool(name="vtiles", bufs=4) as vpool,
            tc.tile_pool(name="outt", bufs=4) as opool,
            tc.tile_pool(name="cps", bufs=4, space="PSUM") as cps_pool,
            tc.tile_pool(name="hps", bufs=2, space="PSUM") as hps_pool,
            tc.tile_pool(name="gps", bufs=2, space="PSUM") as gps_pool,
        ):
            # ---- constants / weights --------------------------------------
            # Sync: zpad (warm-up + halos), then cw diagonals interleaved
            # with the pair-0/1 x tiles.  GpSimd SWDGE: small constants and
            # the h/g weights (its Q7 is idle until the first rest-block).
            # Scalar issues no DMAs at all.
            wz_sb = wpool.tile([P, CH], BF16, tag="wz")
            nc.sync.dma_start(out=wz_sb, in_=zpad[:, :])
            cw_sb = [wpool.tile([P, NTAPS * P], BF16, tag=f"cw{dt}",
                                name=f"cw{dt}")
                     for dt in range(NDT)]

            gbn_sb = [wpool.tile([P, 1], F32, tag=f"gbn{ot}", name=f"gbn{ot}")
                      for ot in range(NOT)]
            kpack_sb = wpool.tile([16, 256], F32, tag="kpack")
            masks_sb = wpool.tile([P, 2], U32, tag="masks")
            hwT_sb = [wpool.tile([P, O], BF16, tag=f"hwT{dt}", name=f"hwT{dt}")
                      for dt in range(NDT)]
            gwT_sb = [wpool.tile([P, O], BF16, tag=f"gwT{dt}", name=f"gwT{dt}")
                      for dt in range(NDT)]

            c_sb = [None] * NCH          # [chunk] -> list of 4 SBUF c tiles
            prev_out = [None] * NOT      # previous chunk's out tile per o-tile
            all_ott = [[None] * NOT for _ in range(NCH)]  # for late stores
            hrow = [None] * NCH          # [chunk] -> [2, CH] copy of h rows 0:2
            # h row 1 packed so partition q holds l in [q*256, (q+1)*256):
            # the final channel-1 store is then one DMA of 16 x 1KB
            # contiguous runs (no HBM read-modify-write penalty)
            hpack = wpool.tile([16, 256], F32, tag="hpack")

            def load_xt_pair(pr, dt):
                # one DMA covers both chunks of the pair: cols
                # [pr*1024-2, pr*1024+1026) of x, halo zero-filled from zpad
                lo = pr * CH2
                xt = xpool.tile([P, CH2 + 4], BF16, tag="xt")
                if pr == 0:
                    nc.sync.dma_start(out=xt[:, 0:2], in_=zpad[:, 0:2])
                    nc.sync.dma_start(out=xt[:, 2:CH2 + 4],
                                      in_=x[dt * P:(dt + 1) * P, 0:CH2 + 2])
                elif pr == NPAIR - 1:
                    nc.sync.dma_start(out=xt[:, CH2 + 2:CH2 + 4],
                                      in_=zpad[:, 0:2])
                    nc.sync.dma_start(out=xt[:, 0:CH2 + 2],
                                      in_=x[dt * P:(dt + 1) * P,
                                            lo - 2:lo + CH2])
                else:
                    nc.sync.dma_start(out=xt[:, :],
                                      in_=x[dt * P:(dt + 1) * P,
                                            lo - 2:lo + CH2 + 2])
                return xt

            def emit_conv_pair(pr, cw_dmas=()):
                """conv for chunks (2*pr, 2*pr+1), tap-paired across chunks."""
                i0, i1 = 2 * pr, 2 * pr + 1
                cw_dmas = list(cw_dmas)
                xts = []
                for dt in range(NDT):
                    if cw_dmas:
                        cw_dmas.pop(0)()
                    xts.append(load_xt_pair(pr, dt))
                tap_order = (2, 0, 1, 3, 4)
                tiles0, tiles1 = [], []
                for dt in range(NDT):
                    cp0 = cps_pool.tile([P, CH], F32, tag="cps")
                    cp1 = cps_pool.tile([P, CH], F32, tag="cps")
                    for j, k in enumerate(tap_order):
                        lw = cw_sb[dt][:, k * P:(k + 1) * P]
                        nc.tensor.matmul(cp0, lhsT=lw,
                                         rhs=xts[dt][:, k:k + CH],
                                         start=(j == 0), stop=(j == NTAPS - 1))
                        nc.tensor.matmul(cp1, lhsT=lw,
                                         rhs=xts[dt][:, CH + k:CH2 + k],
                                         start=(j == 0), stop=(j == NTAPS - 1))
                    ct0 = cpool.tile([P, CH], BF16, tag="ct")
                    nc.scalar.copy(ct0, cp0)
                    ct1 = cpool.tile([P, CH], BF16, tag="ct")
                    nc.scalar.copy(ct1, cp1)
                    tiles0.append(ct0)
                    tiles1.append(ct1)
                c_sb[i0] = tiles0
                c_sb[i1] = tiles1

            store_q = [nc.sync, nc.gpsimd, nc.scalar]

            def emit_store(i, ot, q):
                lo = i * CH
                ott = all_ott[i][ot]
                if ot == 0:
                    # rows 0/1 are produced by the end-pass / pre-zeroing
                    q.dma_start(out=out[2:P, lo:lo + CH], in_=ott[2:P, :])
                else:
                    q.dma_start(out=out[ot * P:(ot + 1) * P, lo:lo + CH],
                                in_=ott)

            def emit_rest_pair(pr):
                """h/g + activation chain + scan for chunks (2*pr, 2*pr+1).

                Stores for pair pr-1 are issued first (their scans finished
                a pair ago, so the queues never block on them), split
                Sync/GpSimd; the final pair's stores are issued inline,
                split across all three queues."""
                i0, i1 = 2 * pr, 2 * pr + 1
                last = (pr == NPAIR - 1)
                if pr > 0:
                    for j, (i, ot) in enumerate(
                            [(2 * pr - 2, t) for t in range(NOT)]
                            + [(2 * pr - 1, t) for t in range(NOT)]):
                        emit_store(i, ot, store_q[j % 2])
                for ot in range(NOT):
                    # g before h: the sigmoid chain (ACT) only needs g, so it
                    # starts while the h matmuls are still streaming
                    gp0 = gps_pool.tile([P, CH], F32, tag="gps")
                    gp1 = gps_pool.tile([P, CH], F32, tag="gps")
                    for dt in range(NDT):
                        lw = gwT_sb[dt][:, ot * P:(ot + 1) * P]
                        nc.tensor.matmul(gp0, lhsT=lw, rhs=c_sb[i0][dt],
                                         start=(dt == 0), stop=(dt == NDT - 1))
                        nc.tensor.matmul(gp1, lhsT=lw, rhs=c_sb[i1][dt],
                                         start=(dt == 0), stop=(dt == NDT - 1))
                    hp0 = hps_pool.tile([P, CH], F32, tag="hps")
                    hp1 = hps_pool.tile([P, CH], F32, tag="hps")
                    for dt in range(NDT):
                        lw = hwT_sb[dt][:, ot * P:(ot + 1) * P]
                        nc.tensor.matmul(hp0, lhsT=lw, rhs=c_sb[i0][dt],
                                         start=(dt == 0), stop=(dt == NDT - 1))
                        nc.tensor.matmul(hp1, lhsT=lw, rhs=c_sb[i1][dt],
                                         start=(dt == 0), stop=(dt == NDT - 1))
                    for ci, (i, gp, hp) in enumerate(
                            [(i0, gp0, hp0), (i1, gp1, hp1)]):
                        # a = sigmoid(-(g + bias)) ; z = 1 - a ; v = z * h
                        # the very last chunk runs the chain in 256-col
                        # halves so its serial tail after the final matmul
                        # is half as long
                        halves = ((0, CH),)
                        if last and ci == 1:
                            halves = ((0, CH // 2), (CH // 2, CH))
                        at = apool.tile([P, CH], F32, tag="at")
                        zt = vpool.tile([P, CH], F32, tag="zt")
                        vt = vpool.tile([P, CH], F32, tag="vt")
                        ott = opool.tile([P, CH], F32, tag=f"out{ot}")
                        for lo, hi in halves:
                            nc.scalar.activation(at[:, lo:hi], gp[:, lo:hi],
                                                 AF.Sigmoid,
                                                 bias=gbn_sb[ot], scale=-1.0)
                            nc.gpsimd.tensor_scalar(zt[:, lo:hi], at[:, lo:hi],
                                                    -1.0, 1.0, OP.mult, OP.add)
                            nc.vector.tensor_tensor(vt[:, lo:hi], zt[:, lo:hi],
                                                    hp[:, lo:hi], OP.mult)
                            if lo == 0:
                                init = (0.0 if i == 0
                                        else prev_out[ot][:, CH - 1:CH])
                            else:
                                init = ott[:, lo - 1:lo]
                            nc.vector.tensor_tensor_scan(
                                ott[:, lo:hi], at[:, lo:hi], vt[:, lo:hi],
                                init, OP.mult, OP.add)
                        if ot == 0 and hrow[i] is None:
                            # stash h row 1: hpack[2i+p, j] = h[1, i*512+p*256+j]
                            ht = wpool.tile([2, CH], F32, tag=f"hrow{i}")
                            nc.vector.tensor_copy(ht, hp[0:2, :])
                            nc.gpsimd.dma_start(
                                out=hpack[2 * i:2 * i + 2, :],
                                in_=ht[1:2, :].rearrange("r (p j) -> r p j",
                                                         j=256))
                            hrow[i] = ht
                        all_ott[i][ot] = ott
                        prev_out[ot] = ott
                        if last:
                            emit_store(i, ot, store_q[(2 * ot + ci) % 3])

            def emit_hrow_early(i):
                # h rows 0:2 for chunk i via a tiny 2-row matmul so the
                # end-pass doesn't have to wait for the full h of the last
                # chunks.
                cpx = cps_pool.tile([P, CH], F32, tag="cps", name=f"cpx{i}")
                for dt in range(NDT):
                    nc.tensor.matmul(
                        cpx[0:2, :],
                        lhsT=hwT_sb[dt][:, 0:2],
                        rhs=c_sb[i][dt],
                        start=(dt == 0), stop=(dt == NDT - 1),
                    )
                ht = wpool.tile([2, CH], F32, tag=f"hrow{i}", name=f"hrowE{i}")
                nc.vector.tensor_copy(ht, cpx[0:2, :])
                nc.gpsimd.dma_start(
                    out=hpack[2 * i:2 * i + 2, :],
                    in_=ht[1:2, :].rearrange("r (p j) -> r p j", j=256))
                hrow[i] = ht

            def emit_endpass():
                # ---- channel 1 on the packed [16, 256] tile ----
                # replicates the reference's f32 rounding:
                # out[1,l] = sign(h)*exp(fl(fl(K+ln|h|) - K)), K = 1000(l+1).
                absm = masks_sb[0:16, 0:1]
                sgnm = masks_sb[0:16, 1:2]
                t = wpool.tile([16, 256], F32, tag="ch1w", name="ch1w")
                nc.vector.tensor_scalar(t.bitcast(U32), hpack.bitcast(U32),
                                        absm, None, OP.bitwise_and)
                nc.vector.tensor_scalar_max(t, t, 1e-6)
                nc.scalar.activation(t, t, AF.Ln)
                nc.vector.tensor_tensor(t, t, kpack_sb, OP.add)
                nc.vector.tensor_tensor(t, t, kpack_sb, OP.subtract)
                nc.scalar.activation(t, t, AF.Exp)
                res = wpool.tile([16, 256], F32, tag="ch1r", name="ch1r")
                nc.vector.tensor_scalar(res.bitcast(U32), hpack.bitcast(U32),
                                        sgnm, None, OP.bitwise_and)
                nc.vector.tensor_tensor(res.bitcast(U32), res.bitcast(U32),
                                        t.bitcast(U32), OP.bitwise_or)
                # row 1 in one DMA: partition q -> out[1, q*256:(q+1)*256]
                nc.sync.dma_start(
                    out=out[1:2, :].rearrange("r (q j) -> r q j", j=256),
                    in_=res)

            # ---- prologue: warm-up + conv pair 0 --------------------------
            # PE warm-up: dummy matmuls on the zero tile during the initial
            # DMA wait trip the HAM clock gate to 2.4 GHz before real work
            # arrives; a few [2,512] streams keep it busy until x lands.
            wps = cps_pool.tile([P, CH], F32, tag="cps", name="warmps")
            for _ in range(28):
                nc.tensor.matmul(wps[0:2, 0:2], lhsT=wz_sb[:, 0:2],
                                 rhs=wz_sb[:, 0:2], start=True, stop=True)
            for _ in range(3):
                nc.tensor.matmul(wps[0:2, :], lhsT=wz_sb[:, 0:2],
                                 rhs=wz_sb, start=True, stop=True)
            wout = wpool.tile([2, 2], F32, tag="warmout")
            nc.vector.tensor_copy(wout, wps[0:2, 0:2])

            def _dma_cw(dt):
                return lambda: nc.sync.dma_start(
                    out=cw_sb[dt], in_=cwdiag[dt * P:(dt + 1) * P, :])

            # small constants + h/g weights on the SWDGE queue
            for ot in range(NOT):
                nc.gpsimd.dma_start(out=gbn_sb[ot],
                                    in_=gbn[ot * P:(ot + 1) * P, :])
            nc.gpsimd.dma_start(out=kpack_sb, in_=kpack[:, :])
            nc.gpsimd.dma_start(out=masks_sb, in_=masks[:, :])

            emit_conv_pair(0, cw_dmas=[_dma_cw(0), _dma_cw(1),
                                       _dma_cw(2), _dma_cw(3)])

            for dt in range(NDT):
                nc.gpsimd.dma_start(out=gwT_sb[dt],
                                    in_=gwT[dt * P:(dt + 1) * P, :])
                nc.gpsimd.dma_start(out=hwT_sb[dt],
                                    in_=hwT[dt * P:(dt + 1) * P, :])
            nc.gpsimd.dma_start(out=out[2:4, 0:2], in_=wout)

            # ---- main pipeline, one conv pair ahead -----------------------
            emit_conv_pair(1)
            emit_rest_pair(0)
            emit_conv_pair(2)
            emit_rest_pair(1)
            emit_conv_pair(3)
            emit_hrow_early(NCH - 2)
            emit_hrow_early(NCH - 1)
            emit_rest_pair(2)
            emit_endpass()
            emit_rest_pair(3)

    nc.finalize()
    return nc


_PROGRAM = None


def _get_program():
    global _PROGRAM
    if _PROGRAM is None:
        _PROGRAM = build_program()
    return _PROGRAM


def _bf16(a):
    # round-to-nearest-even f32 -> bf16, returned as uint16-packed bfloat16
    import ml_dtypes
    return np.asarray(a, np.float32).astype(ml_dtypes.bfloat16)


def prepare_in_maps(x, conv_w, h_w, g_w):
    x = np.ascontiguousarray(np.asarray(x), dtype=np.float32)
    conv_w = np.asarray(conv_w, dtype=np.float32)
    h_w = np.asarray(h_w, dtype=np.float32)
    g_w = np.asarray(g_w, dtype=np.float32)

    hwT = np.ascontiguousarray(h_w[:, :, 0].T)                    # [D, O]
    gw_pad = np.zeros((O, D), np.float32)
    gw_pad[2:, :] = g_w[:, :, 0]
    gwT = np.ascontiguousarray(gw_pad.T)                          # [D, O]

    # 5 diagonal matrices per d-tile, concatenated along free dim: [D, 5*128]
    cwdiag = np.zeros((D, NTAPS * P), np.float32)
    for dt in range(NDT):
        for k in range(NTAPS):
            blk = cwdiag[dt * P:(dt + 1) * P, k * P:(k + 1) * P]
            np.fill_diagonal(blk, conv_w[dt * P:(dt + 1) * P, 0, k])

    gbp = np.zeros((O, 1), np.float32)
    gbp[0, 0], gbp[1, 0] = -1000.0, 1000.0
    gbn = np.ascontiguousarray(-gbp)

    # K for the packed layout: kpack[q, j] = 1000*(q*256 + j + 1)
    q = np.arange(16)[:, None]
    j = np.arange(256)[None, :]
    kpack = np.ascontiguousarray(
        (1000.0 * (q * 256 + j + 1.0)).astype(np.float32))        # [16, 256]

    zpad = np.zeros((P, CH), np.float32)
    masks = np.ascontiguousarray(np.broadcast_to(
        np.array([[0x7FFFFFFF, 0x80000000]], np.uint32), (P, 2)))
    xb = _bf16(x)
    return [
        {"x": np.ascontiguousarray(xb[b]), "hwT": _bf16(hwT),
         "gwT": _bf16(gwT), "cwdiag": _bf16(cwdiag),
         "gbn": gbn, "kpack": kpack, "zpad": _bf16(zpad), "masks": masks}
        for b in range(B)
    ]


def kernel(x, conv_w, h_w, g_w):
    in_maps = prepare_in_maps(x, conv_w, h_w, g_w)
    nc = _get_program()
    res = run_bass_kernel_spmd(nc, in_maps, list(range(N_CORES))).results
    return np.stack([res[b]["out"] for b in range(B)], axis=0)


# revision 23
# speedup vs baseline: 1.0241x; 1.0197x over previous
"""MinGRU Trainium2 kernel (v3 — bf16 inputs + schedule-optimized).

Reference computation (per batch b):
    c = depthwise_conv1d(x, conv_w, taps=5, pad=2)        # [D, L]
    h = h_w @ c                                           # [O, L]
    g = concat([-1000, +1000], g_w @ c)                   # [O, L]
    a = sigmoid(-g); v = sigmoid(g) * h
    out[l] = a[l] * out[l-1] + v[l]     (linear scan along L)

Strategy: pure data-parallel over B (8 batches -> 8 NeuronCores).
Per core, everything streams in PAIRS of 512-wide l-chunks:
  - x / conv diagonals / h_w / g_w / c are bf16 (f32 PSUM accumulate;
    measured end-to-end rel err ~4e-3 vs the 2e-2 budget); activations,
    scan and output stay f32.  bf16 also enables the PE fast-weight-load
    path, so LDWEIGHTS fully hides under the matmul stream.
  - conv: 5 diagonal-matmuls per d-tile on TensorE accumulating in PSUM,
    taps interleaved across the pair's two chunks (one x DMA covers the
    whole pair per d-tile)
  - c PSUM->SBUF copies (cast to bf16) on ScalarE; h/g matmuls
    dt-interleaved across the pair so each stationary weight serves two
    512-col streams
  - a = sigmoid(-(g+bias)) on ScalarE (bias carries the +/-1000 rows)
  - z = 1 - a on GpSimd, v = z*h on VectorE, scan via tensor_tensor_scan
  - DMA issue cost is ~0.6us of sequencer time per dma_start, so:
    Sync issues x loads + half the stores, GpSimd (SWDGE) issues
    weights + small constants + the other half of the stores, Scalar
    issues nothing (its sequencer is saturated by copies + sigmoids).
    Stores are issued one pair late so they never head-of-line block;
    the last pair's stores are split across all three queues.
  - channel 0 output is exactly 0 (output buffers are pre-zeroed)
  - channel 1 replicates the reference's f32 log-domain quantization
    out[1,l] = sign(h)*exp(fl(fl(K_l+ln|h|)-K_l)), K_l = 1000(l+1),
    via a packed [128,64] tile (see emit_endpass).
"""

import numpy as np

import concourse.bass as bass
import concourse.mybir as mybir
from concourse import bacc
from concourse.tile import TileContext
from concourse.bass_utils import run_bass_kernel_spmd

F32 = mybir.dt.float32
BF16 = mybir.dt.bfloat16
U32 = mybir.dt.uint32
AF = mybir.ActivationFunctionType
OP = mybir.AluOpType

B, D, O, L = 8, 512, 512, 4096
P = 128
CH = 512                 # l-chunk width (one PSUM bank)
CH2 = 2 * CH             # pair width
NCH = L // CH            # 8
NPAIR = NCH // 2         # 4
NDT = D // P             # 4 d-tiles
NOT = O // P             # 4 o-tiles
NTAPS = 5
N_CORES = 8
PK = CH // 64            # 8 packed columns per chunk


def build_program():
    nc = bacc.Bacc()

    x = nc.declare_dram_parameter("x", [D, L], BF16, isOutput=False)
    hwT = nc.declare_dram_parameter("hwT", [D, O], BF16, isOutput=False)
    gwT = nc.declare_dram_parameter("gwT", [D, O], BF16, isOutput=False)
    cwdiag = nc.declare_dram_parameter("cwdiag", [D, NTAPS * P], BF16,
                                       isOutput=False)
    gbn = nc.declare_dram_parameter("gbn", [O, 1], F32, isOutput=False)
    kpack = nc.declare_dram_parameter("kpack", [16, 256], F32, isOutput=False)
    zpad = nc.declare_dram_parameter("zpad", [P, CH], BF16, isOutput=False)
    masks = nc.declare_dram_parameter("masks", [P, 2], U32, isOutput=False)
    out = nc.declare_dram_parameter("out", [O, L], F32, isOutput=True)

    with TileContext(nc) as tc:
        with (
            tc.tile_pool(name="weights", bufs=1) as wpool,
            tc.tile_pool(name="xin", bufs=8) as xpool,
            tc.tile_pool(name="csb", bufs=16) as cpool,
            tc.tile_pool(name="actout", bufs=6) as apool,
            tc.tile_pool(name="vtiles", bufs=4) as vpool,
            tc.tile_pool(name="outt", bufs=4) as opool,
            tc.tile_pool(name="cps", bufs=4, space="PSUM") as cps_pool,
            tc.tile_pool(name="hps", bufs=2, space="PSUM") as hps_pool,
            tc.tile_pool(name="gps", bufs=2, space="PSUM") as gps_pool,
        ):
            # ---- constants / weights --------------------------------------
            # Sync: zpad (warm-up + halos), then cw diagonals interleaved
            # with the pair-0/1 x tiles.  GpSimd SWDGE: small constants and
            # the h/g weights (its Q7 is idle until the first rest-block).
            # Scalar issues no DMAs at all.
            wz_sb = wpool.tile([P, CH], BF16, tag="wz")
            nc.sync.dma_start(out=wz_sb, in_=zpad[:, :])
            cw_sb = [wpool.tile([P, NTAPS * P], BF16, tag=f"cw{dt}",
                                name=f"cw{dt}")
                     for dt in range(NDT)]

            gbn_sb = [wpool.tile([P, 1], F32, tag=f"gbn{ot}", name=f"gbn{ot}")
                      for ot in range(NOT)]
            kpack_sb = wpool.tile([16, 256], F32, tag="kpack")
            masks_sb = wpool.tile([P, 2], U32, tag="masks")
            hwT_sb = [wpool.tile([P, O], BF16, tag=f"hwT{dt}", name=f"hwT{dt}")
                      for dt in range(NDT)]
            gwT_sb = [wpool.tile([P, O], BF16, tag=f"gwT{dt}", name=f"gwT{dt}")
                      for dt in range(NDT)]

            c_sb = [None] * NCH          # [chunk] -> list of 4 SBUF c tiles
            prev_out = [None] * NOT      # previous chunk's out tile per o-tile
            all_ott = [[None] * NOT for _ in range(NCH)]  # for late stores
            hrow = [None] * NCH          # [chunk] -> [2, CH] copy of h rows 0:2
            # h row 1 packed so partition q holds l in [q*256, (q+1)*256):
            # the final channel-1 store is then one DMA of 16 x 1KB
            # contiguous runs (no HBM read-modify-write penalty)
            hpack = wpool.tile([16, 256], F32, tag="hpack")

            def load_xt_pair(pr, dt):
                # one DMA covers both chunks of the pair: cols
                # [pr*1024-2, pr*1024+1026) of x, halo zero-filled from zpad
                lo = pr * CH2
                xt = xpool.tile([P, CH2 + 4], BF16, tag="xt")
                if pr == 0:
                    nc.sync.dma_start(out=xt[:, 0:2], in_=zpad[:, 0:2])
                    nc.sync.dma_start(out=xt[:, 2:CH2 + 4],
                                      in_=x[dt * P:(dt + 1) * P, 0:CH2 + 2])
                elif pr == NPAIR - 1:
                    nc.sync.dma_start(out=xt[:, CH2 + 2:CH2 + 4],
                                      in_=zpad[:, 0:2])
                    nc.sync.dma_start(out=xt[:, 0:CH2 + 2],
                                      in_=x[dt * P:(dt + 1) * P,
                                            lo - 2:lo + CH2])
                else:
                    nc.sync.dma_start(out=xt[:, :],
                                      in_=x[dt * P:(dt + 1) * P,
                                            lo - 2:lo + CH2 + 2])
                return xt

            def emit_conv_pair(pr, cw_dmas=()):
                """conv for chunks (2*pr, 2*pr+1), tap-paired across chunks."""
                i0, i1 = 2 * pr, 2 * pr + 1
                cw_dmas = list(cw_dmas)
                xts = []
                for dt in range(NDT):
                    if cw_dmas:
                        cw_dmas.pop(0)()
                    xts.append(load_xt_pair(pr, dt))
                tap_order = (2, 0, 1, 3, 4)
                tiles0, tiles1 = [], []
                for dt in range(NDT):
                    cp0 = cps_pool.tile([P, CH], F32, tag="cps")
                    cp1 = cps_pool.tile([P, CH], F32, tag="cps")
                    for j, k in enumerate(tap_order):
                        lw = cw_sb[dt][:, k * P:(k + 1) * P]
                        nc.tensor.matmul(cp0, lhsT=lw,
                                         rhs=xts[dt][:, k:k + CH],
                                         start=(j == 0), stop=(j == NTAPS - 1))
                        nc.tensor.matmul(cp1, lhsT=lw,
                                         rhs=xts[dt][:, CH + k:CH2 + k],
                                         start=(j == 0), stop=(j == NTAPS - 1))
                    ct0 = cpool.tile([P, CH], BF16, tag="ct")
                    nc.scalar.copy(ct0, cp0)
                    ct1 = cpool.tile([P, CH], BF16, tag="ct")
                    nc.scalar.copy(ct1, cp1)
                    tiles0.append(ct0)
                    tiles1.append(ct1)
                c_sb[i0] = tiles0
                c_sb[i1] = tiles1

            store_q = [nc.sync, nc.gpsimd, nc.scalar]

            def emit_store(i, ot, q):
                lo = i * CH
                ott = all_ott[i][ot]
                if ot == 0:
                    # rows 0/1 are produced by the end-pass / pre-zeroing
                    q.dma_start(out=out[2:P, lo:lo + CH], in_=ott[2:P, :])
                else:
                    q.dma_start(out=out[ot * P:(ot + 1) * P, lo:lo + CH],
                                in_=ott)

            def emit_rest_pair(pr):
                """h/g + activation chain + scan for chunks (2*pr, 2*pr+1).

                Stores for pair pr-1 are issued first (their scans finished
                a pair ago, so the queues never block on them), split
                Sync/GpSimd; the final pair's stores are issued inline,
                split across all three queues."""
                i0, i1 = 2 * pr, 2 * pr + 1
                last = (pr == NPAIR - 1)
                if pr > 0:
                    for j, (i, ot) in enumerate(
                            [(2 * pr - 2, t) for t in range(NOT)]
                            + [(2 * pr - 1, t) for t in range(NOT)]):
                        emit_store(i, ot, store_q[j % 2])
                for ot in range(NOT):
                    # g before h: the sigmoid chain (ACT) only needs g, so it
                    # starts while the h matmuls are still streaming
                    gp0 = gps_pool.tile([P, CH], F32, tag="gps")
                    gp1 = gps_pool.tile([P, CH], F32, tag="gps")
                    for dt in range(NDT):
                        lw = gwT_sb[dt][:, ot * P:(ot + 1) * P]
                        nc.tensor.matmul(gp0, lhsT=lw, rhs=c_sb[i0][dt],
                                         start=(dt == 0), stop=(dt == NDT - 1))
                        nc.tensor.matmul(gp1, lhsT=lw, rhs=c_sb[i1][dt],
                                         start=(dt == 0), stop=(dt == NDT - 1))
                    hp0 = hps_pool.tile([P, CH], F32, tag="hps")
                    hp1 = hps_pool.tile([P, CH], F32, tag="hps")
                    for dt in range(NDT):
                        lw = hwT_sb[dt][:, ot * P:(ot + 1) * P]
                        nc.tensor.matmul(hp0, lhsT=lw, rhs=c_sb[i0][dt],
                                         start=(dt == 0), stop=(dt == NDT - 1))
                        nc.tensor.matmul(hp1, lhsT=lw, rhs=c_sb[i1][dt],
                                         start=(dt == 0), stop=(dt == NDT - 1))
                    for ci, (i, gp, hp) in enumerate(
                            [(i0, gp0, hp0), (i1, gp1, hp1)]):
                        # a = sigmoid(-(g + bias)) ; z = 1 - a ; v = z * h
                        at = apool.tile([P, CH], F32, tag="at")
                        nc.scalar.activation(at, gp, AF.Sigmoid,
                                             bias=gbn_sb[ot], scale=-1.0)
                        zt = vpool.tile([P, CH], F32, tag="zt")
                        nc.gpsimd.tensor_scalar(zt, at, -1.0, 1.0,
                                                OP.mult, OP.add)
                        vt = vpool.tile([P, CH], F32, tag="vt")
                        nc.vector.tensor_tensor(vt, zt, hp, OP.mult)
                        ott = opool.tile([P, CH], F32, tag=f"out{ot}")
                        init = 0.0 if i == 0 else prev_out[ot][:, CH - 1:CH]
                        nc.vector.tensor_tensor_scan(ott, at, vt, init,
                                                     OP.mult, OP.add)
                        if ot == 0 and hrow[i] is None:
                            # stash h row 1: hpack[2i+p, j] = h[1, i*512+p*256+j]
                            ht = wpool.tile([2, CH], F32, tag=f"hrow{i}")
                            nc.vector.tensor_copy(ht, hp[0:2, :])
                            nc.gpsimd.dma_start(
                                out=hpack[2 * i:2 * i + 2, :],
                                in_=ht[1:2, :].rearrange("r (p j) -> r p j",
                                                         j=256))
                            hrow[i] = ht
                        all_ott[i][ot] = ott
                        prev_out[ot] = ott
                        if last:
                            emit_store(i, ot, store_q[(2 * ot + ci) % 3])

            def emit_hrow_early(i):
                # h rows 0:2 for chunk i via a tiny 2-row matmul so the
                # end-pass doesn't have to wait for the full h of the last
                # chunks.
                cpx = cps_pool.tile([P, CH], F32, tag="cps", name=f"cpx{i}")
                for dt in range(NDT):
                    nc.tensor.matmul(
                        cpx[0:2, :],
                        lhsT=hwT_sb[dt][:, 0:2],
                        rhs=c_sb[i][dt],
                        start=(dt == 0), stop=(dt == NDT - 1),
                    )
                ht = wpool.tile([2, CH], F32, tag=f"hrow{i}", name=f"hrowE{i}")
                nc.vector.tensor_copy(ht, cpx[0:2, :])
                nc.gpsimd.dma_start(
                    out=hpack[2 * i:2 * i + 2, :],
                    in_=ht[1:2, :].rearrange("r (p j) -> r p j", j=256))
                hrow[i] = ht

            def emit_endpass():
                # ---- channel 1 on the packed [16, 256] tile ----
                # replicates the reference's f32 rounding:
                # out[1,l] = sign(h)*exp(fl(fl(K+ln|h|) - K)), K = 1000(l+1).
                absm = masks_sb[0:16, 0:1]
                sgnm = masks_sb[0:16, 1:2]
                t = wpool.tile([16, 256], F32, tag="ch1w", name="ch1w")
                nc.vector.tensor_scalar(t.bitcast(U32), hpack.bitcast(U32),
                                        absm, None, OP.bitwise_and)
                nc.vector.tensor_scalar_max(t, t, 1e-6)
                nc.scalar.activation(t, t, AF.Ln)
                nc.vector.tensor_tensor(t, t, kpack_sb, OP.add)
                nc.vector.tensor_tensor(t, t, kpack_sb, OP.subtract)
                nc.scalar.activation(t, t, AF.Exp)
                res = wpool.tile([16, 256], F32, tag="ch1r", name="ch1r")
                nc.vector.tensor_scalar(res.bitcast(U32), hpack.bitcast(U32),
                                        sgnm, None, OP.bitwise_and)
                nc.vector.tensor_tensor(res.bitcast(U32), res.bitcast(U32),
                                        t.bitcast(U32), OP.bitwise_or)
                # row 1 in one DMA: partition q -> out[1, q*256:(q+1)*256]
                nc.sync.dma_start(
                    out=out[1:2, :].rearrange("r (q j) -> r q j", j=256),
                    in_=res)

            # ---- prologue: warm-up + conv pair 0 --------------------------
            # PE warm-up: dummy matmuls on the zero tile during the initial
            # DMA wait trip the HAM clock gate to 2.4 GHz before real work
            # arrives; a few [2,512] streams keep it busy until x lands.
            wps = cps_pool.tile([P, CH], F32, tag="cps", name="warmps")
            for _ in range(28):
                nc.tensor.matmul(wps[0:2, 0:2], lhsT=wz_sb[:, 0:2],
                                 rhs=wz_sb[:, 0:2], start=True, stop=True)
            for _ in range(3):
                nc.tensor.matmul(wps[0:2, :], lhsT=wz_sb[:, 0:2],
                                 rhs=wz_sb, start=True, stop=True)
            wout = wpool.tile([2, 2], F32, tag="warmout")
            nc.vector.tensor_copy(wout, wps[0:2, 0:2])

            def _dma_cw(dt):
                return lambda: nc.sync.dma_start(
                    out=cw_sb[dt], in_=cwdiag[dt * P:(dt + 1) * P, :])

            # small constants + h/g weights on the SWDGE queue
            for ot in range(NOT):
                nc.gpsimd.dma_start(out=gbn_sb[ot],
                                    in_=gbn[ot * P:(ot + 1) * P, :])
            nc.gpsimd.dma_start(out=kpack_sb, in_=kpack[:, :])
            nc.gpsimd.dma_start(out=masks_sb, in_=masks[:, :])

            emit_conv_pair(0, cw_dmas=[_dma_cw(0), _dma_cw(1),
                                       _dma_cw(2), _dma_cw(3)])

            for dt in range(NDT):
                nc.gpsimd.dma_start(out=gwT_sb[dt],
                                    in_=gwT[dt * P:(dt + 1) * P, :])
                nc.gpsimd.dma_start(out=hwT_sb[dt],
                                    in_=hwT[dt * P:(dt + 1) * P, :])
            nc.gpsimd.dma_start(out=out[2:4, 0:2], in_=wout)

            # ---- main pipeline, one conv pair ahead -----------------------
            emit_conv_pair(1)
            emit_rest_pair(0)
            emit_conv_pair(2)
            emit_rest_pair(1)
            emit_conv_pair(3)
            emit_rest_pair(2)
            emit_rest_pair(3)
            # end-pass last: its ACT table reloads (sigmoid->ln->exp) land in
            # the tail where ScalarE is idle, and the single channel-1 store
            # hides under the final out-store drain
            emit_endpass()

    nc.finalize()
    return nc


_PROGRAM = None


def _get_program():
    global _PROGRAM
    if _PROGRAM is None:
        _PROGRAM = build_program()
    return _PROGRAM


def _bf16(a):
    # round-to-nearest-even f32 -> bf16, returned as uint16-packed bfloat16
    import ml_dtypes
    return np.asarray(a, np.float32).astype(ml_dtypes.bfloat16)


def prepare_in_maps(x, conv_w, h_w, g_w):
    x = np.ascontiguousarray(np.asarray(x), dtype=np.float32)
    conv_w = np.asarray(conv_w, dtype=np.float32)
    h_w = np.asarray(h_w, dtype=np.float32)
    g_w = np.asarray(g_w, dtype=np.float32)

    hwT = np.ascontiguousarray(h_w[:, :, 0].T)                    # [D, O]
    gw_pad = np.zeros((O, D), np.float32)
    gw_pad[2:, :] = g_w[:, :, 0]
    gwT = np.ascontiguousarray(gw_pad.T)                          # [D, O]

    # 5 diagonal matrices per d-tile, concatenated along free dim: [D, 5*128]
    cwdiag = np.zeros((D, NTAPS * P), np.float32)
    for dt in range(NDT):
        for k in range(NTAPS):
            blk = cwdiag[dt * P:(dt + 1) * P, k * P:(k + 1) * P]
            np.fill_diagonal(blk, conv_w[dt * P:(dt + 1) * P, 0, k])

    gbp = np.zeros((O, 1), np.float32)
    gbp[0, 0], gbp[1, 0] = -1000.0, 1000.0
    gbn = np.ascontiguousarray(-gbp)

    # K for the packed layout: kpack[q, j] = 1000*(q*256 + j + 1)
    q = np.arange(16)[:, None]
    j = np.arange(256)[None, :]
    kpack = np.ascontiguousarray(
        (1000.0 * (q * 256 + j + 1.0)).astype(np.float32))        # [16, 256]

    zpad = np.zeros((P, CH), np.float32)
    masks = np.ascontiguousarray(np.broadcast_to(
        np.array([[0x7FFFFFFF, 0x80000000]], np.uint32), (P, 2)))
    xb = _bf16(x)
    return [
        {"x": np.ascontiguousarray(xb[b]), "hwT": _bf16(hwT),
         "gwT": _bf16(gwT), "cwdiag": _bf16(cwdiag),
         "gbn": gbn, "kpack": kpack, "zpad": _bf16(zpad), "masks": masks}
        for b in range(B)
    ]


def kernel(x, conv_w, h_w, g_w):
    in_maps = prepare_in_maps(x, conv_w, h_w, g_w)
    nc = _get_program()
    res = run_bass_kernel_spmd(nc, in_maps, list(range(N_CORES))).results
    return np.stack([res[b]["out"] for b in range(B)], axis=0)
